# revision 1
# baseline (speedup 1.0000x reference)
"""EvoformerBlock kernel for 8 Trainium2 NeuronCores.

Strategy: run the full block under jax.jit on the axon-tunneled neuron
devices. MSA tensor is sharded over the N (sequence) dim and pair tensors
over the first L dim where XLA's SPMD partitioner can do so profitably;
the partitioner inserts the all-gathers required by the triangular
updates (full k-dim contraction), matching the sequence-parallel /
replicated-triangle hint. Falls back to single-device and then CPU
execution if the sharded compile is unsupported by the backend.
"""

import numpy as np

import jax
import jax.numpy as jnp
from functools import partial

N, L, CM, CZ, H = 128, 192, 256, 128, 8
EPS = 1e-5


def _ln(x, g, b):
    mu = x.mean(-1, keepdims=True)
    var = x.var(-1, keepdims=True)
    return (x - mu) / jnp.sqrt(var + EPS) * g + b


def _lin(x, W, b=None):
    y = x @ W
    return y + b if b is not None else y


def _msa_row_attn(msa, pair, p):
    n, l, _ = msa.shape
    d = CM // H
    m = _ln(msa, p['ln_m_g'], p['ln_m_b'])
    z = _ln(pair, p['ln_z_g'], p['ln_z_b'])
    q = _lin(m, p['Wq']).reshape(n, l, H, d)
    k = _lin(m, p['Wk']).reshape(n, l, H, d)
    v = _lin(m, p['Wv']).reshape(n, l, H, d)
    a = jnp.einsum('bihd,bjhd->bhij', q, k) / (d ** 0.5)
    a = a + _lin(z, p['Wpb']).transpose(2, 0, 1)[None]
    a = jax.nn.softmax(a, axis=-1)
    out = jnp.einsum('bhij,bjhd->bihd', a, v).reshape(n, l, CM)
    return msa + jax.nn.sigmoid(_lin(m, p['Wg'], p['bg'])) * _lin(out, p['Wo'], p['bo'])


def _transition(x, p):
    h = jax.nn.relu(_lin(_ln(x, p['ln_g'], p['ln_b']), p['W1'], p['b1']))
    return x + _lin(h, p['W2'], p['b2'])


def _opm(msa, p):
    m = _ln(msa, p['ln_g'], p['ln_b'])
    left = _lin(m, p['Wl'], p['bl'])
    right = _lin(m, p['Wr'], p['br'])
    outer = jnp.einsum('sic,sjd->ijcd', left, right) / msa.shape[0]
    l = outer.shape[0]
    return _lin(outer.reshape(l, l, -1), p['Wout'], p['bout'])


def _tri_mult(pair, p, mode):
    z = _ln(pair, p['ln_g'], p['ln_b'])
    left = _lin(z, p['Wlp'], p['blp']) * jax.nn.sigmoid(_lin(z, p['Wlg'], p['blg']))
    right = _lin(z, p['Wrp'], p['brp']) * jax.nn.sigmoid(_lin(z, p['Wrg'], p['brg']))
    if mode == 'outgoing':
        out = jnp.einsum('ikc,jkc->ijc', left, right)
    else:
        out = jnp.einsum('kic,kjc->ijc', left, right)
    out = _lin(_ln(out, p['fn_g'], p['fn_b']), p['Wop'], p['bop'])
    return pair + jax.nn.sigmoid(_lin(pair, p['Wog'], p['bog'])) * out


def _tri_attn(pair, p, mode):
    if mode == 'ending':
        pair = pair.transpose(1, 0, 2)
    l = pair.shape[0]
    d = CZ // H
    z = _ln(pair, p['ln_g'], p['ln_b'])
    q = _lin(z, p['Wq']).reshape(l, l, H, d)
    k = _lin(z, p['Wk']).reshape(l, l, H, d)
    v = _lin(z, p['Wv']).reshape(l, l, H, d)
    a = jnp.einsum('ijhd,ikhd->hijk', q, k) / (d ** 0.5)
    a = a + _lin(z, p['Wb']).transpose(2, 0, 1)[:, None]
    a = jax.nn.softmax(a, axis=-1)
    out = jnp.einsum('hijk,ikhd->ijhd', a, v).reshape(l, l, CZ)
    res = pair + jax.nn.sigmoid(_lin(pair, p['Wg'], p['bg'])) * _lin(out, p['Wo'], p['bo'])
    if mode == 'ending':
        res = res.transpose(1, 0, 2)
    return res


def _forward(msa, pair, params):
    msa = _msa_row_attn(msa, pair, params['msa_attn'])
    msa = _transition(msa, params['msa_trans'])
    pair = pair + _opm(msa, params['opm'])
    pair = _tri_mult(pair, params['tri_out'], 'outgoing')
    pair = _tri_mult(pair, params['tri_in'], 'incoming')
    pair = _tri_attn(pair, params['tri_s'], 'starting')
    pair = _tri_attn(pair, params['tri_e'], 'ending')
    pair = _transition(pair, params['pair_trans'])
    return msa, pair


_COMPILED = {}


def _get_fn(kind):
    """kind: 'sharded' | 'single' | 'cpu'. Returns a jitted forward."""
    if kind in _COMPILED:
        return _COMPILED[kind]
    if kind == 'cpu':
        cpu = jax.devices('cpu')[0]
        fn = jax.jit(_forward, device=cpu)
    elif kind == 'single':
        dev = jax.devices()[0]
        fn = jax.jit(_forward, device=dev)
    else:  # sharded over 8 cores
        from jax.sharding import Mesh, NamedSharding, PartitionSpec as P
        devs = np.array(jax.devices()[:8])
        mesh = Mesh(devs, ('x',))
        s_msa = NamedSharding(mesh, P('x', None, None))
        s_pair = NamedSharding(mesh, P('x', None, None))
        s_rep = NamedSharding(mesh, P())
        def rep_tree(t):
            return jax.tree_util.tree_map(lambda _: s_rep, t)
        fn = jax.jit(
            _forward,
            in_shardings=(s_msa, s_pair, None),
            out_shardings=(s_msa, s_pair),
        )
    _COMPILED[kind] = fn
    return fn


def kernel(msa, pair, params):
    msa = jnp.asarray(msa, jnp.float32)
    pair = jnp.asarray(pair, jnp.float32)
    params = jax.tree_util.tree_map(lambda x: jnp.asarray(x, jnp.float32), params)

    for kind in ('sharded', 'single', 'cpu'):
        try:
            fn = _get_fn(kind)
            out_msa, out_pair = fn(msa, pair, params)
            out_msa = np.asarray(jax.device_get(out_msa), np.float32)
            out_pair = np.asarray(jax.device_get(out_pair), np.float32)
            return out_msa, out_pair
        except Exception as e:  # noqa: BLE001 — fall through to next backend
            import traceback, sys
            print(f"[kernel] {kind} path failed: {e}", file=sys.stderr)
            traceback.print_exc()
            continue
    raise RuntimeError("all execution paths failed")


# revision 3
# speedup vs baseline: 1.0496x; 1.0496x over previous
"""EvoformerBlock kernel for 8 Trainium2 NeuronCores.

Strategy: run the full block under jax.jit on the axon-tunneled neuron
devices. MSA tensor is sharded over the N (sequence) dim and pair tensors
over the first L dim where XLA's SPMD partitioner can do so profitably;
the partitioner inserts the all-gathers required by the triangular
updates (full k-dim contraction), matching the sequence-parallel /
replicated-triangle hint. Falls back to single-device and then CPU
execution if the sharded compile is unsupported by the backend.
"""

import numpy as np

import jax
import jax.numpy as jnp
from functools import partial

N, L, CM, CZ, H = 128, 192, 256, 128, 8
EPS = 1e-5


def _ln(x, g, b):
    mu = x.mean(-1, keepdims=True)
    var = x.var(-1, keepdims=True)
    return (x - mu) / jnp.sqrt(var + EPS) * g + b


_BF = jnp.bfloat16


def _mm(a, b):
    """bf16 matmul with fp32 accumulation — 4x faster than fp32 on the PE."""
    return jnp.matmul(a.astype(_BF), b.astype(_BF),
                      preferred_element_type=jnp.float32)


def _ein(spec, a, b):
    return jnp.einsum(spec, a.astype(_BF), b.astype(_BF),
                      preferred_element_type=jnp.float32)


def _lin(x, W, b=None):
    y = _mm(x, W)
    return y + b if b is not None else y


def _msa_row_attn(msa, pair, p):
    n, l, _ = msa.shape
    d = CM // H
    m = _ln(msa, p['ln_m_g'], p['ln_m_b'])
    z = _ln(pair, p['ln_z_g'], p['ln_z_b'])
    q = _lin(m, p['Wq']).reshape(n, l, H, d)
    k = _lin(m, p['Wk']).reshape(n, l, H, d)
    v = _lin(m, p['Wv']).reshape(n, l, H, d)
    a = _ein('bihd,bjhd->bhij', q, k) / (d ** 0.5)
    a = a + _lin(z, p['Wpb']).transpose(2, 0, 1)[None]
    a = jax.nn.softmax(a, axis=-1)
    out = _ein('bhij,bjhd->bihd', a, v).reshape(n, l, CM)
    return msa + jax.nn.sigmoid(_lin(m, p['Wg'], p['bg'])) * _lin(out, p['Wo'], p['bo'])


def _transition(x, p):
    h = jax.nn.relu(_lin(_ln(x, p['ln_g'], p['ln_b']), p['W1'], p['b1']))
    return x + _lin(h, p['W2'], p['b2'])


def _opm(msa, p):
    m = _ln(msa, p['ln_g'], p['ln_b'])
    left = _lin(m, p['Wl'], p['bl'])
    right = _lin(m, p['Wr'], p['br'])
    outer = _ein('sic,sjd->ijcd', left, right) / msa.shape[0]
    l = outer.shape[0]
    return _lin(outer.reshape(l, l, -1), p['Wout'], p['bout'])


def _tri_mult(pair, p, mode):
    z = _ln(pair, p['ln_g'], p['ln_b'])
    left = _lin(z, p['Wlp'], p['blp']) * jax.nn.sigmoid(_lin(z, p['Wlg'], p['blg']))
    right = _lin(z, p['Wrp'], p['brp']) * jax.nn.sigmoid(_lin(z, p['Wrg'], p['brg']))
    if mode == 'outgoing':
        out = _ein('ikc,jkc->ijc', left, right)
    else:
        out = _ein('kic,kjc->ijc', left, right)
    out = _lin(_ln(out, p['fn_g'], p['fn_b']), p['Wop'], p['bop'])
    return pair + jax.nn.sigmoid(_lin(pair, p['Wog'], p['bog'])) * out


def _tri_attn(pair, p, mode):
    if mode == 'ending':
        pair = pair.transpose(1, 0, 2)
    l = pair.shape[0]
    d = CZ // H
    z = _ln(pair, p['ln_g'], p['ln_b'])
    q = _lin(z, p['Wq']).reshape(l, l, H, d)
    k = _lin(z, p['Wk']).reshape(l, l, H, d)
    v = _lin(z, p['Wv']).reshape(l, l, H, d)
    a = _ein('ijhd,ikhd->hijk', q, k) / (d ** 0.5)
    a = a + _lin(z, p['Wb']).transpose(2, 0, 1)[:, None]
    a = jax.nn.softmax(a, axis=-1)
    out = _ein('hijk,ikhd->ijhd', a, v).reshape(l, l, CZ)
    res = pair + jax.nn.sigmoid(_lin(pair, p['Wg'], p['bg'])) * _lin(out, p['Wo'], p['bo'])
    if mode == 'ending':
        res = res.transpose(1, 0, 2)
    return res


def _forward(msa, pair, params):
    msa = _msa_row_attn(msa, pair, params['msa_attn'])
    msa = _transition(msa, params['msa_trans'])
    pair = pair + _opm(msa, params['opm'])
    pair = _tri_mult(pair, params['tri_out'], 'outgoing')
    pair = _tri_mult(pair, params['tri_in'], 'incoming')
    pair = _tri_attn(pair, params['tri_s'], 'starting')
    pair = _tri_attn(pair, params['tri_e'], 'ending')
    pair = _transition(pair, params['pair_trans'])
    return msa, pair


_COMPILED = {}


def _get_fn(kind):
    """kind: 'sharded' | 'single' | 'cpu'. Returns a jitted forward."""
    if kind in _COMPILED:
        return _COMPILED[kind]
    if kind == 'cpu':
        cpu = jax.devices('cpu')[0]
        fn = jax.jit(_forward, device=cpu)
    elif kind == 'single':
        dev = jax.devices()[0]
        fn = jax.jit(_forward, device=dev)
    else:  # sharded over 8 cores
        from jax.sharding import Mesh, NamedSharding, PartitionSpec as P
        devs = np.array(jax.devices()[:8])
        mesh = Mesh(devs, ('x',))
        s_msa = NamedSharding(mesh, P('x', None, None))
        s_pair = NamedSharding(mesh, P('x', None, None))
        s_rep = NamedSharding(mesh, P())
        def rep_tree(t):
            return jax.tree_util.tree_map(lambda _: s_rep, t)
        fn = jax.jit(
            _forward,
            in_shardings=(s_msa, s_pair, None),
            out_shardings=(s_msa, s_pair),
        )
    _COMPILED[kind] = fn
    return fn


def kernel(msa, pair, params):
    msa = jnp.asarray(msa, jnp.float32)
    pair = jnp.asarray(pair, jnp.float32)
    params = jax.tree_util.tree_map(lambda x: jnp.asarray(x, jnp.float32), params)

    for kind in ('sharded', 'single', 'cpu'):
        try:
            fn = _get_fn(kind)
            out_msa, out_pair = fn(msa, pair, params)
            out_msa = np.asarray(jax.device_get(out_msa), np.float32)
            out_pair = np.asarray(jax.device_get(out_pair), np.float32)
            return out_msa, out_pair
        except Exception as e:  # noqa: BLE001 — fall through to next backend
            import traceback, sys
            print(f"[kernel] {kind} path failed: {e}", file=sys.stderr)
            traceback.print_exc()
            continue
    raise RuntimeError("all execution paths failed")
